# revision 23
# baseline (speedup 1.0000x reference)
"""GCN encoder (GAE-style) on 8 TRN2 NeuronCores via Bass.

Math (PyG GCNConv, self-loops, symmetric norm), with A' = (A+I) and
D = diag(deg^-1/2):
    h  = relu( D A' D (x @ W1) + b1 )
    mu = D A' D (h @ Wmu) + bmu ;  logvar = ... Wlv ... blv

norm = dinv[src]*dinv[dst] factorizes: the src factor rides in the gather
table (x pre-scaled by D on host), the dst factor rides in the one-hot
scatter matrix S.  Self-loops are ordinary edges (src=dst).  mu/logvar
share everything: Wf = [Wmu|Wlv], out = [mu|logvar].

Per core (dst rows sharded, 12544 rows each over N'=100352):
  setup:  g1 = (D x) @ W1 for ALL nodes (replicated compute) -> local HBM
  SpMM A: edges sorted by (src-window, dst-block), padded per cell to 128;
          dma_gather g1[src] in <=896-idx chunks (descriptor-ring limit),
          DVE builds S[e,d] = (iota==dst_rel[e])*dinv_dst[e] in one op,
          PE: psum[d,f] += S^T @ M per 128-edge tile, psum flushed into an
          SBUF accumulator per (window,block) cell.
  h/HS:   HS = dinv * relu(acc + b1)
  g2:     PE-transpose HS tiles, g2 = HS @ Wf, AllGather -> g2_full
  SpMM B: identical structure on g2_full, out = acc2 + bf -> HBM out
"""
import contextlib
import numpy as np
import sys

sys.path.insert(0, "/opt/trn_rl_repo")
sys.path.insert(0, "/root/.axon_site/_ro/trn_rl_repo")

N = 100_000
Z = 64
NCORES = 8
NPAD = 100_352            # 8 * 12544, multiple of 128
ROWS = NPAD // NCORES     # 12544 local dst rows per core
NBLK = ROWS // 128        # 98 dst blocks per core
WIN = 32_768              # int16 gather window
NWIN = 4
CHUNK = 896               # idxs per dma_gather, ring-safe at depth 2
MSG_DEPTH = 3
S_DEPTH = 16
PSUM_SLOTS = (0, 4, 8, 12, 16, 20, 24, 28)  # one 128x128 slot per PSUM bank
SET_CH = 1024             # setup rows per xT stage chunk
NSET = NPAD // SET_CH


def _build_layout(src, dst):
    loop = np.arange(N, dtype=np.int64)
    src = np.concatenate([src.astype(np.int64), loop])
    dst = np.concatenate([dst.astype(np.int64), loop])

    deg = np.bincount(dst, minlength=NPAD).astype(np.float64)
    dinv = np.zeros(NPAD, np.float32)
    nz = deg > 0
    dinv[nz] = (1.0 / np.sqrt(deg[nz])).astype(np.float32)

    core = dst // ROWS
    w = src // WIN
    b = (dst % ROWS) // 128

    idx3 = (core * NWIN + w) * NBLK + b
    counts = np.bincount(idx3, minlength=NCORES * NWIN * NBLK).reshape(
        NCORES, NWIN, NBLK
    )
    T = np.maximum(1, np.ceil(counts.max(axis=0) / 128).astype(np.int64))

    cell_sizes = T * 128                       # [NWIN, NBLK]
    flat = cell_sizes.reshape(-1)
    cell_off = np.zeros((NWIN, NBLK), np.int64)
    cell_off.reshape(-1)[1:] = np.cumsum(flat)[:-1]
    E_slots = int(flat.sum())
    E_slots = ((E_slots + 127) // 128) * 128

    gidx = np.zeros((NCORES, E_slots), np.int16)
    dstrel = np.full((NCORES, E_slots), -1.0, np.float32)
    dinvdst = np.zeros((NCORES, E_slots), np.float32)

    order = np.lexsort((b, w, core))
    src_s, dst_s, core_s, w_s, b_s = (
        src[order], dst[order], core[order], w[order], b[order]
    )
    key = (core_s * NWIN + w_s) * NBLK + b_s
    changes = np.empty(len(key), bool)
    changes[0] = True
    np.not_equal(key[1:], key[:-1], out=changes[1:])
    run_start = np.maximum.accumulate(np.where(changes, np.arange(len(key)), 0))
    pos = np.arange(len(key)) - run_start

    slot = cell_off[w_s, b_s] + pos
    gidx[core_s, slot] = (src_s - w_s * WIN).astype(np.int16)
    dstrel[core_s, slot] = (dst_s % 128).astype(np.float32)
    dinvdst[core_s, slot] = dinv[dst_s]

    chunks = []            # (start_slot, length, window)
    for wi in range(NWIN):
        wstart = int(cell_off[wi, 0])
        wlen = int(cell_sizes[wi].sum())
        p = wstart
        while p < wstart + wlen:
            ln = min(CHUNK, wstart + wlen - p)
            chunks.append((p, ln, wi))
            p += ln
    slot2chunk = np.empty(E_slots, np.int64)
    for ci, (s, ln, wi) in enumerate(chunks):
        slot2chunk[s : s + ln] = ci

    tiles = []             # (slot, chunk_id, cell_id, first, last)
    cells = []             # (w, b, ntiles)
    cid = 0
    for wi in range(NWIN):
        for bi in range(NBLK):
            nt = int(T[wi, bi])
            cells.append((wi, bi, nt))
            for t in range(nt):
                s = int(cell_off[wi, bi]) + t * 128
                tiles.append((s, int(slot2chunk[s]), cid, t == 0, t == nt - 1))
            cid += 1

    return {
        "dinv": dinv, "gidx": gidx, "dstrel": dstrel, "dinvdst": dinvdst,
        "E_slots": E_slots, "chunks": chunks, "tiles": tiles, "cells": cells,
    }


def _wrap16(a):
    C, E = a.shape
    w = a.reshape(C, E // 16, 16).transpose(0, 2, 1)
    return np.tile(w, (1, 8, 1)).copy()


def _wrap128(a):
    C, E = a.shape
    return a.reshape(C, E // 128, 128).transpose(0, 2, 1).copy()


def _build_program(E_slots, chunks, tiles, cells):
    from concourse import bass, bacc
    from concourse import mybir

    f32 = mybir.dt.float32
    bf16 = mybir.dt.bfloat16
    i16 = mybir.dt.int16
    nc = bacc.Bacc(None, target_bir_lowering=False)

    xT = nc.dram_tensor("xT", [128, NPAD], f32, kind="ExternalInput")
    w1 = nc.dram_tensor("w1", [128, 128], f32, kind="ExternalInput")
    wf = nc.dram_tensor("wf", [128, 128], f32, kind="ExternalInput")
    b1b = nc.dram_tensor("b1b", [128, 128], f32, kind="ExternalInput")
    bfb = nc.dram_tensor("bfb", [128, 128], f32, kind="ExternalInput")
    gidx_d = nc.dram_tensor("gidx", [128, E_slots // 16], i16, kind="ExternalInput")
    dstrel_d = nc.dram_tensor("dstrel", [128, E_slots // 128], f32, kind="ExternalInput")
    dinvdst_d = nc.dram_tensor("dinvdst", [128, E_slots // 128], f32, kind="ExternalInput")
    dinvloc_d = nc.dram_tensor("dinvloc", [128, NBLK], f32, kind="ExternalInput")
    out_d = nc.dram_tensor("out", [ROWS, 128], f32, kind="ExternalOutput")

    g1 = nc.dram_tensor("g1_full", [NPAD, 128], bf16)
    cc_in = nc.dram_tensor("cc_in", [ROWS, 128], bf16)
    g2 = nc.dram_tensor("g2_full", [NPAD, 128], bf16, addr_space="Shared")

    win_last_setchunk = [
        min(NSET - 1, ((w + 1) * WIN - 1) // SET_CH) for w in range(NWIN)
    ]
    TPC = SET_CH // 128    # tiles per setup chunk (16)

    es = contextlib.ExitStack()
    with es:
        block = es.enter_context(nc.Block())
        sems = {}
        for name in ("init", "xin", "meta", "pe_set", "dve_set", "g1w",
                     "gsA0", "gsA1", "gsB0", "gsB1", "msgfA", "msgfB", "sbA", "sbB",
                     "sfA", "sfB", "cdA", "cdB", "cfA", "cfB", "hsr",
                     "tpr", "hstc", "g2r", "g2c", "g2d", "ccs", "outr",
                     "outd"):
            sems[name] = es.enter_context(nc.semaphore(name))
        (init_sem, xin_sem, meta_sem, pe_set, dve_set, g1w_sem, gsA0, gsA1,
         gsB0, gsB1, msgfA, msgfB, sbA, sbB, sfA, sfB, cdA, cdB, cfA, cfB,
         hsr, tpr, hstc, g2r, g2c, g2d, cc_sem, outr, outd) = (
            sems[n] for n in ("init", "xin", "meta", "pe_set", "dve_set",
                              "g1w", "gsA0", "gsA1", "gsB0", "gsB1",
                              "msgfA", "msgfB",
                              "sbA", "sbB", "sfA", "sfB", "cdA", "cdB",
                              "cfA", "cfB", "hsr", "tpr", "hstc", "g2r",
                              "g2c", "g2d", "ccs", "outr", "outd"))
        sb = lambda name, shape, dt: es.enter_context(nc.sbuf_tensor(name, shape, dt))
        iota_t = sb("iota", [128, 128], f32)
        pcol = sb("pcol", [128, 1], f32)
        w1s = sb("w1s", [128, 128], f32)
        wfs = sb("wfs", [128, 128], f32)
        b1s = sb("b1s", [128, 128], f32)
        bfs = sb("bfs", [128, 128], f32)
        ident = sb("ident", [128, 128], f32)
        dinvloc = sb("dinvloc_s", [128, NBLK], f32)
        gidx_t = sb("gidx_t", [128, E_slots // 16], i16)
        dstrel = sb("dstrel_s", [128, E_slots // 128], f32)
        dinvdst = sb("dinvdst_s", [128, E_slots // 128], f32)
        msg = sb("msg", [128, MSG_DEPTH, CHUNK // 128, 128], bf16)
        S_t = sb("sbuf_S", [128, S_DEPTH, 128], bf16)
        g1st = sb("g1st", [128, 2 * SET_CH], bf16)
        g2st = sb("g2st", [128, NBLK * 128], bf16)
        acc = sb("acc", [128, NBLK, 128], f32)
        hs = sb("hs", [128, NBLK, 128], f32)
        hst = sb("hst", [128, 2, 128], f32)
        big = sb("big", [128, 2 * SET_CH], f32)
        ps = es.enter_context(nc.psum_tensor("ps", [128, 32, 128], f32))

        def PS(s):
            return ps[:, s : s + 1, :].rearrange("p a f -> p (a f)")

        # ---------------- SYNC ENGINE ----------------
        @block.sync
        def _(s: bass.BassEngine):
            s.dma_start(out=w1s[:], in_=w1[:]).then_inc(meta_sem, 16)
            s.dma_start(out=wfs[:], in_=wf[:]).then_inc(meta_sem, 16)
            s.dma_start(out=b1s[:], in_=b1b[:]).then_inc(meta_sem, 16)
            s.dma_start(out=bfs[:], in_=bfb[:]).then_inc(meta_sem, 16)
            s.dma_start(out=gidx_t[:], in_=gidx_d[:]).then_inc(meta_sem, 16)
            s.dma_start(out=dstrel[:], in_=dstrel_d[:]).then_inc(meta_sem, 16)
            s.dma_start(out=dinvdst[:], in_=dinvdst_d[:]).then_inc(meta_sem, 16)
            s.dma_start(out=dinvloc[:], in_=dinvloc_d[:]).then_inc(meta_sem, 16)
            for c in range(NSET):
                buf = c % 2
                if c >= 2:
                    s.wait_ge(pe_set, (c - 1) * TPC)   # xT stage consumed
                s.dma_start(
                    out=big[:, buf * SET_CH : (buf + 1) * SET_CH],
                    in_=xT[:, c * SET_CH : (c + 1) * SET_CH],
                ).then_inc(xin_sem, 16)
                s.wait_ge(dve_set, (c + 1) * TPC)
                if c >= 1:
                    s.wait_ge(g1w_sem, 16 * c)
                s.dma_start(
                    out=g1[c * SET_CH : (c + 1) * SET_CH, :].rearrange(
                        "(a b) f -> b a f", b=128
                    ),
                    in_=g1st[:, buf * SET_CH : (buf + 1) * SET_CH].rearrange(
                        "p (a f) -> p a f", f=128
                    ),
                ).then_inc(g1w_sem, 16)
            s.wait_ge(g2c, NBLK)
            s.dma_start(
                out=cc_in[:].rearrange("(a b) f -> b a f", b=128),
                in_=g2st[:].rearrange("p (a f) -> p a f", f=128),
            ).then_inc(g2d, 16)
            s.wait_ge(outr, NBLK)
            s.dma_start(
                out=out_d[:].rearrange("(a b) f -> b a f", b=128),
                in_=hs[:],
            ).then_inc(outd, 16)

        # ---------------- GPSIMD ----------------
        @block.gpsimd
        def _(g: bass.BassEngine):
            g.iota(iota_t[:], pattern=[[1, 128]], base=0, channel_multiplier=0,
                   allow_small_or_imprecise_dtypes=True).then_inc(init_sem, 1)
            g.iota(pcol[:], pattern=[[1, 1]], base=0, channel_multiplier=1,
                   allow_small_or_imprecise_dtypes=True).then_inc(init_sem, 1)
            g.wait_ge(init_sem, 2)
            g.tensor_scalar(ident[:], iota_t[:], pcol[:], None,
                            mybir.AluOpType.is_equal).then_inc(init_sem, 1)
            g.wait_ge(meta_sem, 16 * 8)
            last_cell_of_chunk = {}
            for (sl_, ci_, cell_, fi_, la_) in tiles:
                last_cell_of_chunk[ci_] = max(last_cell_of_chunk.get(ci_, -1), cell_)
            for phase, (gsp, cdp, table) in enumerate(
                (((gsA0, gsA1), cdA, g1), ((gsB0, gsB1), cdB, g2))
            ):
                if phase == 1:
                    g.wait_ge(g2d, 16)
                    g.collective_compute(
                        "AllGather",
                        mybir.AluOpType.bypass,
                        replica_groups=[list(range(NCORES))],
                        ins=[cc_in[:]],
                        outs=[g2[:]],
                    ).then_inc(cc_sem, 1)
                    g.wait_ge(cc_sem, 1)
                for ci, (slot0, ln, wi) in enumerate(chunks):
                    if phase == 0:
                        g.wait_ge(g1w_sem, 16 * (win_last_setchunk[wi] + 1))
                    if ci >= 2:
                        g.wait_ge(gsp[ci % 2], 16 * (ci // 2))
                    if ci >= MSG_DEPTH:
                        g.wait_ge(cdp, last_cell_of_chunk[ci - MSG_DEPTH] + 1)
                    wbase = wi * WIN
                    wsize = min(WIN, NPAD - wbase)
                    g.dma_gather(
                        out_ap=msg[:, ci % MSG_DEPTH, : ln // 128, :],
                        in_ap=table[wbase : wbase + wsize, :],
                        idxs_ap=gidx_t[:, slot0 // 16 : (slot0 + ln) // 16],
                        num_idxs=ln,
                        num_idxs_reg=ln,
                        elem_size=128,
                    ).then_inc(gsp[ci % 2], 16)

        # ---------------- VECTOR ENGINE ----------------
        @block.vector
        def _(v: bass.BassEngine):
            v.wait_ge(init_sem, 3)
            v.wait_ge(meta_sem, 16 * 8)
            for c in range(NSET):
                buf = c % 2
                for t in range(TPC):
                    gt = c * TPC + t
                    v.wait_ge(pe_set, gt + 1)
                    if c >= 2 and t == 0:
                        v.wait_ge(g1w_sem, 16 * (c - 1))  # g1 stage drained
                    v.tensor_copy(
                        out=g1st[:, buf * SET_CH + t * 128 : buf * SET_CH + (t + 1) * 128],
                        in_=PS(PSUM_SLOTS[gt % 4]),
                    ).then_inc(dve_set, 1)
            v.memset(acc[:], 0.0)
            v.drain()
            for phase, (sb, sf, cd, cf) in enumerate(
                ((sbA, sfA, cdA, cfA), (sbB, sfB, cdB, cfB))
            ):
                prev_cell = -1
                for ti, (slot, ci, cell, first, last) in enumerate(tiles):
                    if ti % 4 == 0 and ti + 4 > S_DEPTH:
                        v.wait_ge(sf, min(ti + 4, len(tiles)) - S_DEPTH)
                    col = slot // 128
                    v.tensor_scalar(
                        S_t[:, ti % S_DEPTH, :],
                        iota_t[:],
                        dstrel[:, col : col + 1],
                        dinvdst[:, col : col + 1],
                        mybir.AluOpType.is_equal,
                        mybir.AluOpType.mult,
                    ).then_inc(sb, 1)
                    if last:
                        if prev_cell >= 0:
                            pb = cells[prev_cell][1]
                            v.wait_ge(cd, prev_cell + 1)
                            v.tensor_tensor(
                                out=acc[:, pb, :], in0=acc[:, pb, :],
                                in1=PS(PSUM_SLOTS[prev_cell % 8]),
                                op=mybir.AluOpType.add,
                            ).then_inc(cf, 1)
                        prev_cell = cell
                pb = cells[prev_cell][1]
                v.wait_ge(cd, prev_cell + 1)
                v.tensor_tensor(
                    out=acc[:, pb, :], in0=acc[:, pb, :],
                    in1=PS(PSUM_SLOTS[prev_cell % 8]),
                    op=mybir.AluOpType.add,
                ).then_inc(cf, 1)

                if phase == 0:
                    v.drain()
                    for bi in range(NBLK):
                        v.tensor_tensor(
                            out=hs[:, bi, :], in0=acc[:, bi, :], in1=b1s[:],
                            op=mybir.AluOpType.add,
                        )
                        v.drain()
                        v.tensor_scalar(
                            hs[:, bi, :], hs[:, bi, :],
                            0.0, dinvloc[:, bi : bi + 1],
                            mybir.AluOpType.max, mybir.AluOpType.mult,
                        ).then_inc(hsr, 1)
                    for bi in range(NBLK):
                        v.wait_ge(tpr, bi + 1)
                        v.tensor_copy(
                            out=hst[:, bi % 2, :],
                            in_=PS(PSUM_SLOTS[(2 * bi) % 8]),
                        ).then_inc(hstc, 1)
                        v.wait_ge(g2r, bi + 1)
                        v.tensor_copy(
                            out=g2st[:, bi * 128 : (bi + 1) * 128],
                            in_=PS(PSUM_SLOTS[(2 * bi + 1) % 8]),
                        ).then_inc(g2c, 1)
                    v.memset(acc[:], 0.0)
                    v.drain()
                else:
                    v.wait_ge(g2d, 16)
                    v.drain()
                    for bi in range(NBLK):
                        v.tensor_tensor(
                            out=hs[:, bi, :], in0=acc[:, bi, :], in1=bfs[:],
                            op=mybir.AluOpType.add,
                        ).then_inc(outr, 1)

        # ---------------- TENSOR ENGINE ----------------
        @block.tensor
        def _(t: bass.BassEngine):
            t.wait_ge(meta_sem, 16 * 8)
            for c in range(NSET):
                buf = c % 2
                t.wait_ge(xin_sem, 16 * (c + 1))
                for tt in range(TPC):
                    gt = c * TPC + tt
                    if gt >= 4:
                        t.wait_ge(dve_set, gt - 3)
                    t.matmul(
                            PS(PSUM_SLOTS[gt % 4]),
                            lhsT=big[:, buf * SET_CH + tt * 128 : buf * SET_CH + (tt + 1) * 128],
                            rhs=w1s[:],
                            start=True,
                            stop=True,
                        ).then_inc(pe_set, 1)
            for phase, (gsp, sb, sf, cd, cf) in enumerate(
                (((gsA0, gsA1), sbA, sfA, cdA, cfA),
                 ((gsB0, gsB1), sbB, sfB, cdB, cfB))
            ):
                NT = len(tiles)
                cell_last = {}
                for _ti, (_s, _c, _cell, _f, _l) in enumerate(tiles):
                    cell_last[_cell] = _ti
                ncl = len(cells)
                prev_ci = -1
                need_sb = 0
                for ti, (slot, ci, cell, first, last) in enumerate(tiles):
                    if ci != prev_ci:
                        t.wait_ge(gsp[ci % 2], 16 * (ci // 2 + 1))
                        prev_ci = ci
                    if ti + 1 > need_sb:
                        cap = cell_last[min(cell + 1, ncl - 1)] + 1
                        need_sb = max(ti + 1, min(ti + 4, cap, NT))
                        t.wait_ge(sb, need_sb)
                    if first and cell >= 8:
                        t.wait_ge(cf, cell - 7)
                    sl, ln, wi = chunks[ci]
                    pos = (slot - sl) // 128
                    inst = t.matmul(
                            PS(PSUM_SLOTS[cell % 8]),
                            lhsT=S_t[:, ti % S_DEPTH, :],
                            rhs=msg[:, ci % MSG_DEPTH, pos, :],
                            start=first,
                            stop=last,
                            skip_group_check=True,
                        )
                    inst.then_inc(sf, 1)
                    if last:
                        t.wait_ge(sf, ti + 1).then_inc(cd, 1)
                if phase == 0:
                    t.wait_ge(init_sem, 3)
                    for bi in range(NBLK):
                        t.wait_ge(hsr, bi + 1)
                        if bi >= 4:
                            t.wait_ge(g2c, bi - 3)
                        t.transpose(
                            PS(PSUM_SLOTS[(2 * bi) % 8]),
                            hs[:, bi, :],
                            ident[:],
                        ).then_inc(tpr, 1)
                        t.wait_ge(hstc, bi + 1)
                        t.matmul(
                                PS(PSUM_SLOTS[(2 * bi + 1) % 8]),
                                lhsT=hst[:, bi % 2, :],
                                rhs=wfs[:],
                                start=True,
                                stop=True,
                            ).then_inc(g2r, 1)

    nc.finalize()
    return nc


_CACHE = {}


def kernel(x, edge_index, W1, b1, Wmu, bmu, Wlv, blv):
    x = np.asarray(x, np.float32)
    edge_index = np.asarray(edge_index)
    W1 = np.ascontiguousarray(np.asarray(W1, np.float32))
    Wf = np.ascontiguousarray(np.concatenate(
        [np.asarray(Wmu, np.float32), np.asarray(Wlv, np.float32)], axis=1
    ))
    b1 = np.asarray(b1, np.float32)
    bf = np.concatenate([np.asarray(bmu, np.float32), np.asarray(blv, np.float32)])

    lay = _build_layout(edge_index[0], edge_index[1])
    dinv = lay["dinv"]

    xs = x * dinv[:N, None]
    xT = np.zeros((128, NPAD), np.float32)
    xT[:, :N] = xs.T

    gidx_w = _wrap16(lay["gidx"])
    dstrel_w = _wrap128(lay["dstrel"])
    dinvdst_w = _wrap128(lay["dinvdst"])
    dinvloc = dinv.reshape(NCORES, NBLK, 128).transpose(0, 2, 1).copy()

    b1b = np.tile(b1[None, :], (128, 1)).astype(np.float32)
    bfb = np.tile(bf[None, :], (128, 1)).astype(np.float32)

    key = (lay["E_slots"], len(lay["chunks"]), len(lay["tiles"]))
    if key not in _CACHE:
        _CACHE[key] = _build_program(
            lay["E_slots"], lay["chunks"], lay["tiles"], lay["cells"]
        )
    nc = _CACHE[key]

    in_maps = []
    for c in range(NCORES):
        in_maps.append({
            "xT": xT,
            "w1": W1,
            "wf": Wf,
            "b1b": b1b,
            "bfb": bfb,
            "gidx": gidx_w[c],
            "dstrel": dstrel_w[c],
            "dinvdst": dinvdst_w[c],
            "dinvloc": dinvloc[c],
        })
    results = _run(nc, in_maps)
    out = np.concatenate([results[c]["out"] for c in range(NCORES)], axis=0)[:N]
    return (np.ascontiguousarray(out[:, :Z]), np.ascontiguousarray(out[:, Z:]))


def _make_runner(nc):
    """Compile the SPMD program once; return (call, in_names, out_names)."""
    import jax
    from concourse import bass2jax, mybir
    from jax.experimental.shard_map import shard_map
    from jax.sharding import Mesh, PartitionSpec

    bass2jax.install_neuronx_cc_hook()
    pname = nc.partition_id_tensor.name if nc.partition_id_tensor else None
    in_names, out_names, out_avals, zero_outs = [], [], [], []
    for alloc in nc.m.functions[0].allocations:
        if not isinstance(alloc, mybir.MemoryLocationSet):
            continue
        name = alloc.memorylocations[0].name
        if alloc.kind == "ExternalInput":
            if name != pname:
                in_names.append(name)
        elif alloc.kind == "ExternalOutput":
            shape = tuple(alloc.tensor_shape)
            dt = mybir.dt.np(alloc.dtype)
            out_names.append(name)
            out_avals.append(jax.core.ShapedArray(shape, dt))
            zero_outs.append(np.zeros(shape, dt))
    n_params = len(in_names)
    all_names = in_names + out_names
    if pname is not None:
        all_names = all_names + [pname]

    def _body(*args):
        operands = list(args)
        if pname is not None:
            operands.append(bass2jax.partition_id_tensor())
        outs = bass2jax._bass_exec_p.bind(
            *operands,
            out_avals=tuple(out_avals),
            in_names=tuple(all_names),
            out_names=tuple(out_names),
            lowering_input_output_aliases=(),
            sim_require_finite=True,
            sim_require_nnan=True,
            nc=nc,
        )
        return tuple(outs)

    devices = jax.devices()[:NCORES]
    mesh = Mesh(np.asarray(devices), ("core",))
    nio = n_params + len(out_names)
    sharded = jax.jit(
        shard_map(
            _body, mesh=mesh,
            in_specs=(PartitionSpec("core"),) * nio,
            out_specs=(PartitionSpec("core"),) * len(out_names),
            check_rep=False,
        ),
        keep_unused=True,
    )
    return sharded, in_names, out_names, zero_outs, mesh


def _run(nc, in_maps):
    import jax
    from jax.sharding import NamedSharding, PartitionSpec

    if "runner" not in _CACHE:
        _CACHE["runner"] = _make_runner(nc)
    sharded, in_names, out_names, zero_outs, mesh = _CACHE["runner"]

    concat = [
        np.concatenate([np.asarray(in_maps[c][n]) for c in range(NCORES)], axis=0)
        for n in in_names
    ] + [np.zeros((NCORES * z.shape[0], *z.shape[1:]), z.dtype) for z in zero_outs]
    sh = NamedSharding(mesh, PartitionSpec("core"))
    dev_in = [jax.device_put(a, sh) for a in concat]
    out_arrs = sharded(*dev_in)
    jax.block_until_ready(out_arrs)

    # timed re-runs on pre-staged inputs: executable compiled, inputs on device
    import time
    best = None
    for _ in range(3):
        t0 = time.perf_counter_ns()
        out2 = sharded(*dev_in)
        jax.block_until_ready(out2)
        t1 = time.perf_counter_ns()
        best = t1 - t0 if best is None else min(best, t1 - t0)
    kernel._last_exec_ns = best

    res = [
        {
            n: np.asarray(out_arrs[i]).reshape(NCORES, -1, *out_arrs[i].shape[1:])[c].reshape(
                out_arrs[i].shape[0] // NCORES, *out_arrs[i].shape[1:]
            )
            for i, n in enumerate(out_names)
        }
        for c in range(NCORES)
    ]
    return res


# revision 24
# speedup vs baseline: 1.1112x; 1.1112x over previous
"""GCN encoder (GAE-style) on 8 TRN2 NeuronCores via Bass.

Math (PyG GCNConv, self-loops, symmetric norm), with A' = (A+I) and
D = diag(deg^-1/2):
    h  = relu( D A' D (x @ W1) + b1 )
    mu = D A' D (h @ Wmu) + bmu ;  logvar = ... Wlv ... blv

norm = dinv[src]*dinv[dst] factorizes: the src factor rides in the gather
table (x pre-scaled by D on host), the dst factor rides in the one-hot
scatter matrix S.  Self-loops are ordinary edges (src=dst).  mu/logvar
share everything: Wf = [Wmu|Wlv], out = [mu|logvar].

Per core (dst rows sharded, 12544 rows each over N'=100352):
  setup:  g1 = (D x) @ W1 for ALL nodes (replicated compute) -> local HBM
  SpMM A: edges sorted by (src-window, dst-block), padded per cell to 128;
          dma_gather g1[src] in <=896-idx chunks (descriptor-ring limit),
          DVE builds S[e,d] = (iota==dst_rel[e])*dinv_dst[e] in one op,
          PE: psum[d,f] += S^T @ M per 128-edge tile, psum flushed into an
          SBUF accumulator per (window,block) cell.
  h/HS:   HS = dinv * relu(acc + b1)
  g2:     PE-transpose HS tiles, g2 = HS @ Wf, AllGather -> g2_full
  SpMM B: identical structure on g2_full, out = acc2 + bf -> HBM out
"""
import contextlib
import numpy as np
import sys

sys.path.insert(0, "/opt/trn_rl_repo")
sys.path.insert(0, "/root/.axon_site/_ro/trn_rl_repo")

N = 100_000
Z = 64
NCORES = 8
NPAD = 100_352            # 8 * 12544, multiple of 128
ROWS = NPAD // NCORES     # 12544 local dst rows per core
NBLK = ROWS // 128        # 98 dst blocks per core
WIN = 32_768              # int16 gather window
NWIN = 4
CHUNK = 896               # idxs per dma_gather, ring-safe at depth 2
MSG_DEPTH = 3
S_DEPTH = 16
PSUM_SLOTS = (0, 4, 8, 12, 16, 20, 24, 28)  # one 128x128 slot per PSUM bank
SET_CH = 1024             # setup rows per xT stage chunk
NSET = NPAD // SET_CH


def _build_layout(src, dst):
    loop = np.arange(N, dtype=np.int64)
    src = np.concatenate([src.astype(np.int64), loop])
    dst = np.concatenate([dst.astype(np.int64), loop])

    deg = np.bincount(dst, minlength=NPAD).astype(np.float64)
    dinv = np.zeros(NPAD, np.float32)
    nz = deg > 0
    dinv[nz] = (1.0 / np.sqrt(deg[nz])).astype(np.float32)

    core = dst // ROWS
    w = src // WIN
    b = (dst % ROWS) // 128

    idx3 = (core * NWIN + w) * NBLK + b
    counts = np.bincount(idx3, minlength=NCORES * NWIN * NBLK).reshape(
        NCORES, NWIN, NBLK
    )
    T = np.maximum(1, np.ceil(counts.max(axis=0) / 128).astype(np.int64))

    cell_sizes = T * 128                       # [NWIN, NBLK]
    flat = cell_sizes.reshape(-1)
    cell_off = np.zeros((NWIN, NBLK), np.int64)
    cell_off.reshape(-1)[1:] = np.cumsum(flat)[:-1]
    E_slots = int(flat.sum())
    E_slots = ((E_slots + 127) // 128) * 128

    gidx = np.zeros((NCORES, E_slots), np.int16)
    dstrel = np.full((NCORES, E_slots), -1.0, np.float32)
    dinvdst = np.zeros((NCORES, E_slots), np.float32)

    order = np.lexsort((b, w, core))
    src_s, dst_s, core_s, w_s, b_s = (
        src[order], dst[order], core[order], w[order], b[order]
    )
    key = (core_s * NWIN + w_s) * NBLK + b_s
    changes = np.empty(len(key), bool)
    changes[0] = True
    np.not_equal(key[1:], key[:-1], out=changes[1:])
    run_start = np.maximum.accumulate(np.where(changes, np.arange(len(key)), 0))
    pos = np.arange(len(key)) - run_start

    slot = cell_off[w_s, b_s] + pos
    gidx[core_s, slot] = (src_s - w_s * WIN).astype(np.int16)
    dstrel[core_s, slot] = (dst_s % 128).astype(np.float32)
    dinvdst[core_s, slot] = dinv[dst_s]

    chunks = []            # (start_slot, length, window)
    for wi in range(NWIN):
        wstart = int(cell_off[wi, 0])
        wlen = int(cell_sizes[wi].sum())
        p = wstart
        while p < wstart + wlen:
            ln = min(CHUNK, wstart + wlen - p)
            chunks.append((p, ln, wi))
            p += ln
    slot2chunk = np.empty(E_slots, np.int64)
    for ci, (s, ln, wi) in enumerate(chunks):
        slot2chunk[s : s + ln] = ci

    tiles = []             # (slot, chunk_id, cell_id, first, last)
    cells = []             # (w, b, ntiles)
    cid = 0
    for wi in range(NWIN):
        for bi in range(NBLK):
            nt = int(T[wi, bi])
            cells.append((wi, bi, nt))
            for t in range(nt):
                s = int(cell_off[wi, bi]) + t * 128
                tiles.append((s, int(slot2chunk[s]), cid, t == 0, t == nt - 1))
            cid += 1

    return {
        "dinv": dinv, "gidx": gidx, "dstrel": dstrel, "dinvdst": dinvdst,
        "E_slots": E_slots, "chunks": chunks, "tiles": tiles, "cells": cells,
    }


def _wrap16(a):
    C, E = a.shape
    w = a.reshape(C, E // 16, 16).transpose(0, 2, 1)
    return np.tile(w, (1, 8, 1)).copy()


def _wrap128(a):
    C, E = a.shape
    return a.reshape(C, E // 128, 128).transpose(0, 2, 1).copy()


def _build_program(E_slots, chunks, tiles, cells):
    from concourse import bass, bacc
    from concourse import mybir

    f32 = mybir.dt.float32
    bf16 = mybir.dt.bfloat16
    i16 = mybir.dt.int16
    nc = bacc.Bacc(None, target_bir_lowering=False)

    xT = nc.dram_tensor("xT", [128, NPAD], bf16, kind="ExternalInput")
    w1 = nc.dram_tensor("w1", [128, 128], bf16, kind="ExternalInput")
    wf = nc.dram_tensor("wf", [128, 128], f32, kind="ExternalInput")
    b1b = nc.dram_tensor("b1b", [128, 128], f32, kind="ExternalInput")
    bfb = nc.dram_tensor("bfb", [128, 128], f32, kind="ExternalInput")
    gidx_d = nc.dram_tensor("gidx", [128, E_slots // 16], i16, kind="ExternalInput")
    dstrel_d = nc.dram_tensor("dstrel", [128, E_slots // 128], f32, kind="ExternalInput")
    dinvdst_d = nc.dram_tensor("dinvdst", [128, E_slots // 128], f32, kind="ExternalInput")
    dinvloc_d = nc.dram_tensor("dinvloc", [128, NBLK], f32, kind="ExternalInput")
    out_d = nc.dram_tensor("out", [ROWS, 128], f32, kind="ExternalOutput")

    g1 = nc.dram_tensor("g1_full", [NPAD, 128], bf16)
    cc_in = nc.dram_tensor("cc_in", [ROWS, 128], bf16)
    g2 = nc.dram_tensor("g2_full", [NPAD, 128], bf16, addr_space="Shared")

    win_last_setchunk = [
        min(NSET - 1, ((w + 1) * WIN - 1) // SET_CH) for w in range(NWIN)
    ]
    TPC = SET_CH // 128    # tiles per setup chunk (16)

    es = contextlib.ExitStack()
    with es:
        block = es.enter_context(nc.Block())
        sems = {}
        for name in ("init", "xin", "meta", "pe_set", "dve_set", "g1w",
                     "gsA0", "gsA1", "gsB0", "gsB1", "msgfA", "msgfB", "sbA", "sbB",
                     "sfA", "sfB", "cdA", "cdB", "cfA", "cfB", "hsr",
                     "tpr", "hstc", "g2r", "g2c", "g2d", "ccs", "outr",
                     "outd"):
            sems[name] = es.enter_context(nc.semaphore(name))
        (init_sem, xin_sem, meta_sem, pe_set, dve_set, g1w_sem, gsA0, gsA1,
         gsB0, gsB1, msgfA, msgfB, sbA, sbB, sfA, sfB, cdA, cdB, cfA, cfB,
         hsr, tpr, hstc, g2r, g2c, g2d, cc_sem, outr, outd) = (
            sems[n] for n in ("init", "xin", "meta", "pe_set", "dve_set",
                              "g1w", "gsA0", "gsA1", "gsB0", "gsB1",
                              "msgfA", "msgfB",
                              "sbA", "sbB", "sfA", "sfB", "cdA", "cdB",
                              "cfA", "cfB", "hsr", "tpr", "hstc", "g2r",
                              "g2c", "g2d", "ccs", "outr", "outd"))
        sb = lambda name, shape, dt: es.enter_context(nc.sbuf_tensor(name, shape, dt))
        iota_t = sb("iota", [128, 128], f32)
        pcol = sb("pcol", [128, 1], f32)
        w1s = sb("w1s", [128, 128], bf16)
        wfs = sb("wfs", [128, 128], f32)
        b1s = sb("b1s", [128, 128], f32)
        bfs = sb("bfs", [128, 128], f32)
        ident = sb("ident", [128, 128], f32)
        dinvloc = sb("dinvloc_s", [128, NBLK], f32)
        gidx_t = sb("gidx_t", [128, E_slots // 16], i16)
        dstrel = sb("dstrel_s", [128, E_slots // 128], f32)
        dinvdst = sb("dinvdst_s", [128, E_slots // 128], f32)
        msg = sb("msg", [128, MSG_DEPTH, CHUNK // 128, 128], bf16)
        S_t = sb("sbuf_S", [128, S_DEPTH, 128], bf16)
        g1st = sb("g1st", [128, 2 * SET_CH], bf16)
        g2st = sb("g2st", [128, NBLK * 128], bf16)
        acc = sb("acc", [128, NBLK, 128], f32)
        hs = sb("hs", [128, NBLK, 128], f32)
        hst = sb("hst", [128, 2, 128], f32)
        big = sb("big", [128, 2 * SET_CH], bf16)
        ps = es.enter_context(nc.psum_tensor("ps", [128, 32, 128], f32))

        def PS(s):
            return ps[:, s : s + 1, :].rearrange("p a f -> p (a f)")

        # ---------------- SYNC ENGINE ----------------
        @block.sync
        def _(s: bass.BassEngine):
            s.dma_start(out=w1s[:], in_=w1[:]).then_inc(meta_sem, 16)
            s.dma_start(out=wfs[:], in_=wf[:]).then_inc(meta_sem, 16)
            s.dma_start(out=b1s[:], in_=b1b[:]).then_inc(meta_sem, 16)
            s.dma_start(out=bfs[:], in_=bfb[:]).then_inc(meta_sem, 16)
            s.dma_start(out=gidx_t[:], in_=gidx_d[:]).then_inc(meta_sem, 16)
            s.dma_start(out=dstrel[:], in_=dstrel_d[:]).then_inc(meta_sem, 16)
            s.dma_start(out=dinvdst[:], in_=dinvdst_d[:]).then_inc(meta_sem, 16)
            s.dma_start(out=dinvloc[:], in_=dinvloc_d[:]).then_inc(meta_sem, 16)
            for c in range(NSET):
                buf = c % 2
                if c >= 2:
                    s.wait_ge(pe_set, (c - 1) * TPC)   # xT stage consumed
                s.dma_start(
                    out=big[:, buf * SET_CH : (buf + 1) * SET_CH],
                    in_=xT[:, c * SET_CH : (c + 1) * SET_CH],
                ).then_inc(xin_sem, 16)
                s.wait_ge(dve_set, (c + 1) * TPC)
                if c >= 1:
                    s.wait_ge(g1w_sem, 16 * c)
                s.dma_start(
                    out=g1[c * SET_CH : (c + 1) * SET_CH, :].rearrange(
                        "(a b) f -> b a f", b=128
                    ),
                    in_=g1st[:, buf * SET_CH : (buf + 1) * SET_CH].rearrange(
                        "p (a f) -> p a f", f=128
                    ),
                ).then_inc(g1w_sem, 16)
            s.wait_ge(g2c, NBLK)
            s.dma_start(
                out=cc_in[:].rearrange("(a b) f -> b a f", b=128),
                in_=g2st[:].rearrange("p (a f) -> p a f", f=128),
            ).then_inc(g2d, 16)
            s.wait_ge(outr, NBLK)
            s.dma_start(
                out=out_d[:].rearrange("(a b) f -> b a f", b=128),
                in_=hs[:],
            ).then_inc(outd, 16)

        # ---------------- GPSIMD ----------------
        @block.gpsimd
        def _(g: bass.BassEngine):
            g.iota(iota_t[:], pattern=[[1, 128]], base=0, channel_multiplier=0,
                   allow_small_or_imprecise_dtypes=True).then_inc(init_sem, 1)
            g.iota(pcol[:], pattern=[[1, 1]], base=0, channel_multiplier=1,
                   allow_small_or_imprecise_dtypes=True).then_inc(init_sem, 1)
            g.wait_ge(init_sem, 2)
            g.tensor_scalar(ident[:], iota_t[:], pcol[:], None,
                            mybir.AluOpType.is_equal).then_inc(init_sem, 1)
            g.wait_ge(meta_sem, 16 * 8)
            last_cell_of_chunk = {}
            for (sl_, ci_, cell_, fi_, la_) in tiles:
                last_cell_of_chunk[ci_] = max(last_cell_of_chunk.get(ci_, -1), cell_)
            for phase, (gsp, cdp, table) in enumerate(
                (((gsA0, gsA1), cdA, g1), ((gsB0, gsB1), cdB, g2))
            ):
                if phase == 1:
                    g.wait_ge(g2d, 16)
                    g.collective_compute(
                        "AllGather",
                        mybir.AluOpType.bypass,
                        replica_groups=[list(range(NCORES))],
                        ins=[cc_in[:]],
                        outs=[g2[:]],
                    ).then_inc(cc_sem, 1)
                    g.wait_ge(cc_sem, 1)
                for ci, (slot0, ln, wi) in enumerate(chunks):
                    if phase == 0:
                        g.wait_ge(g1w_sem, 16 * (win_last_setchunk[wi] + 1))
                    if ci >= 2:
                        g.wait_ge(gsp[ci % 2], 16 * (ci // 2))
                    if ci >= MSG_DEPTH:
                        g.wait_ge(cdp, last_cell_of_chunk[ci - MSG_DEPTH] + 1)
                    wbase = wi * WIN
                    wsize = min(WIN, NPAD - wbase)
                    g.dma_gather(
                        out_ap=msg[:, ci % MSG_DEPTH, : ln // 128, :],
                        in_ap=table[wbase : wbase + wsize, :],
                        idxs_ap=gidx_t[:, slot0 // 16 : (slot0 + ln) // 16],
                        num_idxs=ln,
                        num_idxs_reg=ln,
                        elem_size=128,
                    ).then_inc(gsp[ci % 2], 16)

        # ---------------- VECTOR ENGINE ----------------
        @block.vector
        def _(v: bass.BassEngine):
            v.wait_ge(init_sem, 3)
            v.wait_ge(meta_sem, 16 * 8)
            for c in range(NSET):
                buf = c % 2
                for t in range(TPC):
                    gt = c * TPC + t
                    v.wait_ge(pe_set, gt + 1)
                    if c >= 2 and t == 0:
                        v.wait_ge(g1w_sem, 16 * (c - 1))  # g1 stage drained
                    v.tensor_copy(
                        out=g1st[:, buf * SET_CH + t * 128 : buf * SET_CH + (t + 1) * 128],
                        in_=PS(PSUM_SLOTS[gt % 4]),
                    ).then_inc(dve_set, 1)
            v.memset(acc[:], 0.0)
            v.drain()
            for phase, (sb, sf, cd, cf) in enumerate(
                ((sbA, sfA, cdA, cfA), (sbB, sfB, cdB, cfB))
            ):
                prev_cell = -1
                for ti, (slot, ci, cell, first, last) in enumerate(tiles):
                    if ti % 4 == 0 and ti + 4 > S_DEPTH:
                        v.wait_ge(sf, min(ti + 4, len(tiles)) - S_DEPTH)
                    col = slot // 128
                    v.tensor_scalar(
                        S_t[:, ti % S_DEPTH, :],
                        iota_t[:],
                        dstrel[:, col : col + 1],
                        dinvdst[:, col : col + 1],
                        mybir.AluOpType.is_equal,
                        mybir.AluOpType.mult,
                    ).then_inc(sb, 1)
                    if last:
                        if prev_cell >= 0:
                            pb = cells[prev_cell][1]
                            v.wait_ge(cd, prev_cell + 1)
                            v.tensor_tensor(
                                out=acc[:, pb, :], in0=acc[:, pb, :],
                                in1=PS(PSUM_SLOTS[prev_cell % 8]),
                                op=mybir.AluOpType.add,
                            ).then_inc(cf, 1)
                        prev_cell = cell
                pb = cells[prev_cell][1]
                v.wait_ge(cd, prev_cell + 1)
                v.tensor_tensor(
                    out=acc[:, pb, :], in0=acc[:, pb, :],
                    in1=PS(PSUM_SLOTS[prev_cell % 8]),
                    op=mybir.AluOpType.add,
                ).then_inc(cf, 1)

                if phase == 0:
                    v.drain()
                    for bi in range(NBLK):
                        v.tensor_tensor(
                            out=hs[:, bi, :], in0=acc[:, bi, :], in1=b1s[:],
                            op=mybir.AluOpType.add,
                        )
                        v.drain()
                        v.tensor_scalar(
                            hs[:, bi, :], hs[:, bi, :],
                            0.0, dinvloc[:, bi : bi + 1],
                            mybir.AluOpType.max, mybir.AluOpType.mult,
                        ).then_inc(hsr, 1)
                    for bi in range(NBLK):
                        v.wait_ge(tpr, bi + 1)
                        v.tensor_copy(
                            out=hst[:, bi % 2, :],
                            in_=PS(PSUM_SLOTS[(2 * bi) % 8]),
                        ).then_inc(hstc, 1)
                        v.wait_ge(g2r, bi + 1)
                        v.tensor_copy(
                            out=g2st[:, bi * 128 : (bi + 1) * 128],
                            in_=PS(PSUM_SLOTS[(2 * bi + 1) % 8]),
                        ).then_inc(g2c, 1)
                    v.memset(acc[:], 0.0)
                    v.drain()
                else:
                    v.wait_ge(g2d, 16)
                    v.drain()
                    for bi in range(NBLK):
                        v.tensor_tensor(
                            out=hs[:, bi, :], in0=acc[:, bi, :], in1=bfs[:],
                            op=mybir.AluOpType.add,
                        ).then_inc(outr, 1)

        # ---------------- TENSOR ENGINE ----------------
        @block.tensor
        def _(t: bass.BassEngine):
            t.wait_ge(meta_sem, 16 * 8)
            for c in range(NSET):
                buf = c % 2
                t.wait_ge(xin_sem, 16 * (c + 1))
                for tt in range(TPC):
                    gt = c * TPC + tt
                    if gt >= 4:
                        t.wait_ge(dve_set, gt - 3)
                    t.matmul(
                            PS(PSUM_SLOTS[gt % 4]),
                            lhsT=big[:, buf * SET_CH + tt * 128 : buf * SET_CH + (tt + 1) * 128],
                            rhs=w1s[:],
                            start=True,
                            stop=True,
                        ).then_inc(pe_set, 1)
            for phase, (gsp, sb, sf, cd, cf) in enumerate(
                (((gsA0, gsA1), sbA, sfA, cdA, cfA),
                 ((gsB0, gsB1), sbB, sfB, cdB, cfB))
            ):
                NT = len(tiles)
                cell_last = {}
                for _ti, (_s, _c, _cell, _f, _l) in enumerate(tiles):
                    cell_last[_cell] = _ti
                ncl = len(cells)
                prev_ci = -1
                need_sb = 0
                for ti, (slot, ci, cell, first, last) in enumerate(tiles):
                    if ci != prev_ci:
                        t.wait_ge(gsp[ci % 2], 16 * (ci // 2 + 1))
                        prev_ci = ci
                    if ti + 1 > need_sb:
                        cap = cell_last[min(cell + 1, ncl - 1)] + 1
                        need_sb = max(ti + 1, min(ti + 4, cap, NT))
                        t.wait_ge(sb, need_sb)
                    if first and cell >= 8:
                        t.wait_ge(cf, cell - 7)
                    sl, ln, wi = chunks[ci]
                    pos = (slot - sl) // 128
                    inst = t.matmul(
                            PS(PSUM_SLOTS[cell % 8]),
                            lhsT=S_t[:, ti % S_DEPTH, :],
                            rhs=msg[:, ci % MSG_DEPTH, pos, :],
                            start=first,
                            stop=last,
                            skip_group_check=True,
                        )
                    inst.then_inc(sf, 1)
                    if last:
                        t.wait_ge(sf, ti + 1).then_inc(cd, 1)
                if phase == 0:
                    t.wait_ge(init_sem, 3)
                    for bi in range(NBLK):
                        t.wait_ge(hsr, bi + 1)
                        if bi >= 4:
                            t.wait_ge(g2c, bi - 3)
                        t.transpose(
                            PS(PSUM_SLOTS[(2 * bi) % 8]),
                            hs[:, bi, :],
                            ident[:],
                        ).then_inc(tpr, 1)
                        t.wait_ge(hstc, bi + 1)
                        t.matmul(
                                PS(PSUM_SLOTS[(2 * bi + 1) % 8]),
                                lhsT=hst[:, bi % 2, :],
                                rhs=wfs[:],
                                start=True,
                                stop=True,
                            ).then_inc(g2r, 1)

    nc.finalize()
    return nc


_CACHE = {}


def kernel(x, edge_index, W1, b1, Wmu, bmu, Wlv, blv):
    x = np.asarray(x, np.float32)
    edge_index = np.asarray(edge_index)
    W1 = np.ascontiguousarray(np.asarray(W1, np.float32))
    Wf = np.ascontiguousarray(np.concatenate(
        [np.asarray(Wmu, np.float32), np.asarray(Wlv, np.float32)], axis=1
    ))
    b1 = np.asarray(b1, np.float32)
    bf = np.concatenate([np.asarray(bmu, np.float32), np.asarray(blv, np.float32)])

    lay = _build_layout(edge_index[0], edge_index[1])
    dinv = lay["dinv"]

    import ml_dtypes
    xs = x * dinv[:N, None]
    xT = np.zeros((128, NPAD), ml_dtypes.bfloat16)
    xT[:, :N] = xs.T.astype(ml_dtypes.bfloat16)

    gidx_w = _wrap16(lay["gidx"])
    dstrel_w = _wrap128(lay["dstrel"])
    dinvdst_w = _wrap128(lay["dinvdst"])
    dinvloc = dinv.reshape(NCORES, NBLK, 128).transpose(0, 2, 1).copy()

    b1b = np.tile(b1[None, :], (128, 1)).astype(np.float32)
    bfb = np.tile(bf[None, :], (128, 1)).astype(np.float32)

    key = (lay["E_slots"], len(lay["chunks"]), len(lay["tiles"]))
    if key not in _CACHE:
        _CACHE[key] = _build_program(
            lay["E_slots"], lay["chunks"], lay["tiles"], lay["cells"]
        )
    nc = _CACHE[key]

    in_maps = []
    for c in range(NCORES):
        in_maps.append({
            "xT": xT,
            "w1": W1.astype(ml_dtypes.bfloat16),
            "wf": Wf,
            "b1b": b1b,
            "bfb": bfb,
            "gidx": gidx_w[c],
            "dstrel": dstrel_w[c],
            "dinvdst": dinvdst_w[c],
            "dinvloc": dinvloc[c],
        })
    results = _run(nc, in_maps)
    out = np.concatenate([results[c]["out"] for c in range(NCORES)], axis=0)[:N]
    return (np.ascontiguousarray(out[:, :Z]), np.ascontiguousarray(out[:, Z:]))


def _make_runner(nc):
    """Compile the SPMD program once; return (call, in_names, out_names)."""
    import jax
    from concourse import bass2jax, mybir
    from jax.experimental.shard_map import shard_map
    from jax.sharding import Mesh, PartitionSpec

    bass2jax.install_neuronx_cc_hook()
    pname = nc.partition_id_tensor.name if nc.partition_id_tensor else None
    in_names, out_names, out_avals, zero_outs = [], [], [], []
    for alloc in nc.m.functions[0].allocations:
        if not isinstance(alloc, mybir.MemoryLocationSet):
            continue
        name = alloc.memorylocations[0].name
        if alloc.kind == "ExternalInput":
            if name != pname:
                in_names.append(name)
        elif alloc.kind == "ExternalOutput":
            shape = tuple(alloc.tensor_shape)
            dt = mybir.dt.np(alloc.dtype)
            out_names.append(name)
            out_avals.append(jax.core.ShapedArray(shape, dt))
            zero_outs.append(np.zeros(shape, dt))
    n_params = len(in_names)
    all_names = in_names + out_names
    if pname is not None:
        all_names = all_names + [pname]

    def _body(*args):
        operands = list(args)
        if pname is not None:
            operands.append(bass2jax.partition_id_tensor())
        outs = bass2jax._bass_exec_p.bind(
            *operands,
            out_avals=tuple(out_avals),
            in_names=tuple(all_names),
            out_names=tuple(out_names),
            lowering_input_output_aliases=(),
            sim_require_finite=True,
            sim_require_nnan=True,
            nc=nc,
        )
        return tuple(outs)

    devices = jax.devices()[:NCORES]
    mesh = Mesh(np.asarray(devices), ("core",))
    nio = n_params + len(out_names)
    sharded = jax.jit(
        shard_map(
            _body, mesh=mesh,
            in_specs=(PartitionSpec("core"),) * nio,
            out_specs=(PartitionSpec("core"),) * len(out_names),
            check_rep=False,
        ),
        keep_unused=True,
    )
    return sharded, in_names, out_names, zero_outs, mesh


def _run(nc, in_maps):
    import jax
    from jax.sharding import NamedSharding, PartitionSpec

    if "runner" not in _CACHE:
        _CACHE["runner"] = _make_runner(nc)
    sharded, in_names, out_names, zero_outs, mesh = _CACHE["runner"]

    concat = [
        np.concatenate([np.asarray(in_maps[c][n]) for c in range(NCORES)], axis=0)
        for n in in_names
    ] + [np.zeros((NCORES * z.shape[0], *z.shape[1:]), z.dtype) for z in zero_outs]
    sh = NamedSharding(mesh, PartitionSpec("core"))
    dev_in = [jax.device_put(a, sh) for a in concat]
    out_arrs = sharded(*dev_in)
    jax.block_until_ready(out_arrs)

    # timed re-runs on pre-staged inputs: executable compiled, inputs on device
    import time
    best = None
    for _ in range(3):
        t0 = time.perf_counter_ns()
        out2 = sharded(*dev_in)
        jax.block_until_ready(out2)
        t1 = time.perf_counter_ns()
        best = t1 - t0 if best is None else min(best, t1 - t0)
    kernel._last_exec_ns = best

    res = [
        {
            n: np.asarray(out_arrs[i]).reshape(NCORES, -1, *out_arrs[i].shape[1:])[c].reshape(
                out_arrs[i].shape[0] // NCORES, *out_arrs[i].shape[1:]
            )
            for i, n in enumerate(out_names)
        }
        for c in range(NCORES)
    ]
    return res
